# revision 1
# baseline (speedup 1.0000x reference)
"""Trainium2 Bass kernel for CLIP-style symmetric contrastive loss.

Problem: image_features [8192, 1024] f32, text_features [8192, 1024] f32.
  loss = 0.5 * (CE(logits, diag) + CE(logits.T, diag)),
  logits = cosine_similarity(img, txt) / 0.07.

Distribution: shard image rows across 8 NeuronCores. Each core m computes the
slab S_m = img_n[m] @ txt_n.T / T  ([1024, 8192]) against the full normalized
text matrix, reduces exp(S - C) along rows (local log-sum-exp) and along
columns (partial column sums), and a single [8194]-float AllReduce combines
the column sums plus the per-core scalar partials. Every core then finishes
the scalar loss locally.

The text matrix ships to the device pre-transposed ([D, N], bf16) so the
contraction dim lands on SBUF partitions with plain contiguous DMAs; its
normalization happens on-device in that layout (ACT squares + PE ones-matmul
partition reduction + per-chunk rsqrt scaling).

Math (C = 1/T upper-bounds every logit, so exp(S - C) <= 1 is stable):
  loss = C + (R + L - (2/T) * Draw) / (2N)
    R    = sum_i log sum_j exp(S_ij - C)
    L    = sum_j log sum_i exp(S_ij - C)
    Draw = sum_i cos(img_i, txt_i)
"""
import threading
from contextlib import ExitStack

import ml_dtypes
import numpy as np

import concourse.bacc as bacc
import concourse.bass as bass
import concourse.bass_isa as bass_isa
import concourse.mybir as mybir
import concourse.tile as tile
from concourse.bass_utils import run_bass_kernel_spmd

F32 = mybir.dt.float32
BF16 = mybir.dt.bfloat16
AF = mybir.ActivationFunctionType
ALU = mybir.AluOpType

N_CORES = 8
N = 8192
D = 1024
TEMPERATURE = 0.07


def build_nc(n=N, d=D, n_cores=N_CORES, no_collective=False, prep_only=False):
    """Build the SPMD Bass program (same program on every core)."""
    inv_t = float(1.0 / TEMPERATURE)
    cexp = float(1.0 / TEMPERATURE)          # stabilizer: max possible logit
    rows = n // n_cores                      # image rows per core
    P = 128
    rp = rows // P                           # row-tiles per core (8)
    kt = d // P                              # contraction tiles (8)
    CH = 512                                 # matmul free-dim chunk
    n_ch = n // CH                           # column chunks (16)
    cb_sz = min(4, n_ch)                     # chunks per psum block

    nc = bacc.Bacc("TRN2", target_bir_lowering=False, debug=False,
                   num_devices=n_cores)
    img = nc.dram_tensor("img", [rows, d], F32, kind="ExternalInput").ap()
    txt_t = nc.dram_tensor("txt_t", [d, n], BF16, kind="ExternalInput").ap()
    txt_own = nc.dram_tensor("txt_own", [rows, d], F32, kind="ExternalInput").ap()
    ones = nc.dram_tensor("ones", [P, P], F32, kind="ExternalInput").ap()
    ones_b = nc.dram_tensor("ones_b", [P, P], BF16, kind="ExternalInput").ap()
    ident = nc.dram_tensor("ident", [P, P], BF16, kind="ExternalInput").ap()
    out = nc.dram_tensor("out", [1, 1], F32, kind="ExternalOutput").ap()

    with tile.TileContext(nc) as tc:
        _body(tc, img, txt_t, txt_own, ones, ones_b, ident, out,
              n=n, d=d, rows=rows, P=P, rp=rp, kt=kt, CH=CH,
              n_ch=n_ch, cb_sz=cb_sz, inv_t=inv_t, cexp=cexp, n_cores=n_cores,
              no_collective=no_collective, prep_only=prep_only)
    nc.compile()
    return nc


def _body(tc, img, txt_t, txt_own, ones, ones_b, ident, out, *, n, d, rows, P,
          rp, kt, CH, n_ch, cb_sz, inv_t, cexp, n_cores, no_collective,
          prep_only):
    nc = tc.nc
    with ExitStack() as ctx:
        persist = ctx.enter_context(tc.tile_pool(name="persist", bufs=1))
        stage_f = ctx.enter_context(tc.tile_pool(name="stage_f", bufs=2))
        stage_b = ctx.enter_context(tc.tile_pool(name="stage_b", bufs=2))
        sqp = ctx.enter_context(tc.tile_pool(name="sqp", bufs=2))
        rbp = ctx.enter_context(tc.tile_pool(name="rbp", bufs=1))
        exp_p = ctx.enter_context(tc.tile_pool(name="exp_p", bufs=4))
        v1 = ctx.enter_context(tc.tile_pool(name="v1", bufs=6))
        csb_p = ctx.enter_context(tc.tile_pool(name="csb_p", bufs=1))
        rpp = ctx.enter_context(tc.tile_pool(name="rpp", bufs=2))
        psum = ctx.enter_context(tc.tile_pool(name="psum", bufs=4, space="PSUM"))
        ssq_ps = ctx.enter_context(tc.tile_pool(name="ssq_ps", bufs=2, space="PSUM"))
        tp_ps = ctx.enter_context(tc.tile_pool(name="tp_ps", bufs=2, space="PSUM"))
        dram = ctx.enter_context(tc.tile_pool(name="dram", bufs=1, space="DRAM"))

        txtT = persist.tile([P, kt, n], BF16, tag="txtT")       # [d-part, k, j]
        imgT = persist.tile([P, kt, rows], BF16, tag="imgT")    # [d-part, k, i]
        acc = persist.tile([P, n], F32, tag="acc")              # col partial sums
        vecs = persist.tile([P, 64], F32, tag="vecs")
        ones_sb = persist.tile([P, P], F32, tag="ones")
        ones_bsb = persist.tile([P, P], BF16, tag="ones_bsb")
        ident_sb = persist.tile([P, P], BF16, tag="ident")
        cs_sb = persist.tile([P, n // P], F32, tag="cs_sb")
        ln_cs = persist.tile([P, n // P], F32, tag="ln_cs")
        ebias = persist.tile([P, 1], F32, tag="ebias")

        cbuf = dram.tile([1, n + 64], F32, tag="cbuf")
        cbuf_out = dram.tile([1, n + 64], F32, tag="cbuf_out", addr_space="Shared")

        nc.sync.dma_start(ones_sb[:], ones[:])
        nc.sync.dma_start(ones_bsb[:], ones_b[:])
        nc.sync.dma_start(ident_sb[:], ident[:])
        nc.gpsimd.memset(ebias[:], float(-cexp))

        # vecs column map:
        RS = 0          # cols 0..rp-1   : per-row-tile rowsum(exp)
        DG = 8          # cols 8..8+rp-1 : per-row-tile diag cosine partials
        LNR = 16        # cols 16..: ln of rowsums
        SC = 56         # col 56: R partial, 57: Draw partial

        # --- Phase A: image prep (+ diag dot with own text rows) ------------
        for t in range(rp):
            img_raw = stage_f.tile([P, d], F32, tag="stage")
            nc.sync.dma_start(img_raw[:], img[t * P:(t + 1) * P, :])
            to_raw = stage_f.tile([P, d], F32, tag="stage")
            nc.sync.dma_start(to_raw[:], txt_own[t * P:(t + 1) * P, :])

            v = v1.tile([P, 8], F32, tag="v1")
            sq = stage_b.tile([P, d], BF16, tag="sq")
            nc.scalar.activation(sq[:], img_raw[:], AF.Square,
                                 accum_out=v[:, 0:1])
            nc.scalar.activation(v[:, 1:2], v[:, 0:1], AF.Sqrt)
            nc.vector.reciprocal(v[:, 2:3], v[:, 1:2])       # 1/||img_i||
            sq2 = stage_b.tile([P, d], BF16, tag="sq")
            nc.scalar.activation(sq2[:], to_raw[:], AF.Square,
                                 accum_out=v[:, 3:4])
            nc.scalar.activation(v[:, 4:5], v[:, 3:4], AF.Sqrt)
            nc.vector.reciprocal(v[:, 5:6], v[:, 4:5])       # 1/||txt_own_i||

            dot_scr = stage_b.tile([P, d], BF16, tag="sq")
            nc.vector.tensor_tensor(dot_scr[:], img_raw[:], to_raw[:], ALU.mult)
            nc.vector.tensor_reduce(v[:, 6:7], dot_scr[:],
                                    axis=mybir.AxisListType.X, op=ALU.add)
            nc.vector.tensor_tensor(v[:, 7:8], v[:, 2:3], v[:, 5:6], ALU.mult)
            nc.vector.tensor_tensor(vecs[:, DG + t:DG + t + 1], v[:, 6:7],
                                    v[:, 7:8], ALU.mult)     # diag cosine

            imgn_b = stage_b.tile([P, d], BF16, tag="nrm")
            nc.vector.tensor_scalar_mul(imgn_b[:], img_raw[:], v[:, 2:3])
            # transpose imgn_b [128 i, 1024 d] into imgT k-tiles via PE
            for k in range(kt):
                tp = tp_ps.tile([P, P], BF16, tag="tp")
                nc.tensor.transpose(tp[:], imgn_b[:, k * P:(k + 1) * P],
                                    ident_sb[:])
                nc.vector.tensor_copy(imgT[:, k, t * P:(t + 1) * P], tp[:])

        # --- Phase B: text load (pre-transposed bf16) + normalize in place --
        for k in range(kt):
            nc.sync.dma_start(txtT[:, k, :], txt_t[k * P:(k + 1) * P, :])
        for c in range(n_ch):
            sl = slice(c * CH, (c + 1) * CH)
            ssq = ssq_ps.tile([P, CH], F32, tag="ssq")
            for k in range(kt):
                sqc = sqp.tile([P, CH], BF16, tag="sqc")
                nc.scalar.activation(sqc[:], txtT[:, k, sl], AF.Square)
                nc.tensor.matmul(ssq[:], ones_bsb[:], sqc[:],
                                 start=(k == 0), stop=(k == kt - 1))
            nrm = rbp.tile([P, CH], BF16, tag="nrm_c")
            nc.scalar.activation(nrm[:], ssq[:], AF.Sqrt)
            rcp = rbp.tile([P, CH], F32, tag="rcp")
            nc.vector.reciprocal(rcp[:], nrm[:])
            rb = rbp.tile([P, CH], BF16, tag="rb")
            nc.vector.tensor_copy(rb[:], rcp[:])
            for k in range(kt):
                nc.vector.tensor_tensor(txtT[:, k, sl], txtT[:, k, sl],
                                        rb[:], ALU.mult)

        if prep_only:
            nc.vector.tensor_reduce(vecs[:, 30:31], txtT[:, 0, 0:CH],
                                    axis=mybir.AxisListType.X, op=ALU.add)
            nc.vector.tensor_reduce(vecs[:, 31:32], imgT[:, 0, 0:CH],
                                    axis=mybir.AxisListType.X, op=ALU.add)
            nc.sync.dma_start(out[0:1, 0:1], vecs[0:1, 30:31])
            return

        # --- Phase C: main matmul + exp + row/col reductions ----------------
        for p in range(rp):
            rparts = rpp.tile([P, n_ch], F32, tag="rp")
            for cb in range(n_ch // cb_sz):
                mms = []
                for _ci in range(cb_sz):
                    mm_t = psum.tile([P, CH], F32, tag="mm")
                    mms.append(mm_t)
                for k in range(kt):
                    for ci in range(cb_sz):
                        c = cb * cb_sz + ci
                        nc.tensor.matmul(
                            mms[ci][:],
                            imgT[:, k, p * P:(p + 1) * P],
                            txtT[:, k, c * CH:(c + 1) * CH],
                            start=(k == 0), stop=(k == kt - 1))
                for ci in range(cb_sz):
                    c = cb * cb_sz + ci
                    ex = exp_p.tile([P, CH], BF16, tag="exp")
                    nc.scalar.activation(ex[:], mms[ci][:], AF.Exp,
                                         bias=ebias[:, 0:1], scale=inv_t,
                                         accum_out=rparts[:, c:c + 1])
                    sl = slice(c * CH, (c + 1) * CH)
                    if p == 0:
                        nc.vector.tensor_copy(acc[:, sl], ex[:])
                    else:
                        nc.vector.tensor_tensor(acc[:, sl], acc[:, sl], ex[:],
                                                ALU.add)
            nc.vector.tensor_reduce(vecs[:, RS + p:RS + p + 1], rparts[:],
                                    axis=mybir.AxisListType.X, op=ALU.add)

        # --- Phase D: local scalars -----------------------------------------
        nc.scalar.activation(vecs[:, LNR:LNR + rp], vecs[:, RS:RS + rp], AF.Ln)
        nc.vector.tensor_reduce(vecs[:, 24:25], vecs[:, LNR:LNR + rp],
                                axis=mybir.AxisListType.X, op=ALU.add)
        nc.gpsimd.partition_all_reduce(vecs[:, SC:SC + 1], vecs[:, 24:25],
                                       channels=P, reduce_op=bass_isa.ReduceOp.add)
        nc.vector.tensor_reduce(vecs[:, 25:26], vecs[:, DG:DG + rp],
                                axis=mybir.AxisListType.X, op=ALU.add)
        nc.gpsimd.partition_all_reduce(vecs[:, SC + 1:SC + 2], vecs[:, 25:26],
                                       channels=P, reduce_op=bass_isa.ReduceOp.add)

        # column partial sums (reduce acc over partitions via ones-matmul)
        for c in range(n_ch):
            ps = psum.tile([P, CH], F32, tag="mm")
            nc.tensor.matmul(ps[:], ones_sb[:], acc[:, c * CH:(c + 1) * CH],
                             start=True, stop=True)
            csb = csb_p.tile([P, CH], F32, tag="csb")
            nc.vector.tensor_copy(csb[0:1, :], ps[0:1, :])
            nc.sync.dma_start(cbuf[0:1, c * CH:(c + 1) * CH], csb[0:1, :])
        nc.sync.dma_start(cbuf[0:1, n:n + 2], vecs[0:1, SC:SC + 2])

        # --- Phase E: AllReduce + finish -------------------------------------
        if no_collective:
            nc.sync.dma_start(cbuf_out[:], cbuf[:])
        else:
            nc.gpsimd.collective_compute(
                "AllReduce", ALU.add,
                replica_groups=[list(range(n_cores))],
                ins=[cbuf[:].opt()], outs=[cbuf_out[:].opt()])

        nc.sync.dma_start(
            cs_sb[:], cbuf_out[0:1, 0:n].rearrange("a (p x) -> (a p) x", p=P))
        nc.scalar.activation(ln_cs[:], cs_sb[:], AF.Ln)
        nc.vector.tensor_reduce(vecs[:, 26:27], ln_cs[:],
                                axis=mybir.AxisListType.X, op=ALU.add)
        nc.gpsimd.partition_all_reduce(vecs[:, 27:28], vecs[:, 26:27],
                                       channels=P, reduce_op=bass_isa.ReduceOp.add)
        rd = v1.tile([P, 8], F32, tag="v1")
        nc.sync.dma_start(rd[0:1, 0:2], cbuf_out[0:1, n:n + 2])

        # loss = cexp + (R + L - (2/T) * Draw) / (2N)
        fin = v1.tile([P, 8], F32, tag="v1")
        nc.vector.tensor_tensor(fin[0:1, 0:1], rd[0:1, 0:1],
                                vecs[0:1, 27:28], ALU.add)          # R + L
        nc.vector.tensor_scalar_mul(fin[0:1, 1:2], rd[0:1, 1:2],
                                    float(-2.0 * inv_t))            # -(2/T) Draw
        nc.vector.tensor_tensor(fin[0:1, 2:3], fin[0:1, 0:1],
                                fin[0:1, 1:2], ALU.add)
        nc.scalar.activation(fin[0:1, 3:4], fin[0:1, 2:3], AF.Copy,
                             bias=float(cexp), scale=float(1.0 / (2 * n)))
        nc.sync.dma_start(out[0:1, 0:1], fin[0:1, 3:4])


def make_in_maps(image_features, text_features, n=N, d=D, n_cores=N_CORES):
    image_features = np.asarray(image_features, dtype=np.float32)
    text_features = np.asarray(text_features, dtype=np.float32)
    rows = n // n_cores
    txt_t = np.ascontiguousarray(text_features.T).astype(ml_dtypes.bfloat16)
    ones = np.ones((128, 128), dtype=np.float32)
    ones_b = np.ones((128, 128), dtype=ml_dtypes.bfloat16)
    ident = np.eye(128, dtype=np.float32).astype(ml_dtypes.bfloat16)
    return [
        {
            "img": image_features[m * rows:(m + 1) * rows],
            "txt_t": txt_t,
            "txt_own": text_features[m * rows:(m + 1) * rows],
            "ones": ones,
            "ones_b": ones_b,
            "ident": ident,
        }
        for m in range(n_cores)
    ]


_CACHE = {}
_LOCK = threading.Lock()


def _get_nc():
    with _LOCK:
        if "nc" not in _CACHE:
            _CACHE["nc"] = build_nc()
        return _CACHE["nc"]


def kernel(image_features, text_features):
    image_features = np.asarray(image_features, dtype=np.float32)
    text_features = np.asarray(text_features, dtype=np.float32)
    assert image_features.shape == (N, D) and text_features.shape == (N, D)
    nc = _get_nc()
    in_maps = make_in_maps(image_features, text_features)
    res = run_bass_kernel_spmd(nc, in_maps, list(range(N_CORES)))
    val = np.float32(res.results[0]["out"][0, 0])
    return np.array(val, dtype=np.float32)



# revision 21
# speedup vs baseline: 1.6734x; 1.6734x over previous
"""Trainium2 Bass kernel for CLIP-style symmetric contrastive loss.

Problem: image_features [8192, 1024] f32, text_features [8192, 1024] f32.
  loss = 0.5 * (CE(logits, diag) + CE(logits.T, diag)),
  logits = cosine_similarity(img, txt) / 0.07.

Distribution: shard image rows across 8 NeuronCores. Each core computes the
transposed slab S^T = txt_raw^T @ img_n  ([8192 txt cols, 1024 img rows]) in
64 chunks of 128 txt columns, using fp8 DoubleRow matmuls (txt raw fp8 as the
stationary operand, normalized img fp8 as the moving operand). The per-txt-col
normalization 1/(T*||txt_j||) folds into the exp activation's per-partition
scale, so the text matrix is never normalized explicitly. Activation
accum_out yields column sums for free; row sums accumulate on the Vector
engine in bf16 and reduce via a ones-matmul.

Per-core text-column norms come from the txt_own diag pass (sum-of-squares of
this core's text rows == this core's 1024 text columns); an early 4 KB
AllGather distributes the reciprocal scales to every core. A single [8194]
AllReduce at the end combines column sums + per-core scalar partials.

Math (C = 1/T upper-bounds every logit, so exp(S - C) <= 1 is stable):
  loss = C + (R + L - 2 * Draw') / (2N)
    R     = sum_i log sum_j exp(S_ij - C)
    L     = sum_j log sum_i exp(S_ij - C)
    Draw' = sum_i cos(img_i, txt_i) / T
"""
import threading
from contextlib import ExitStack

import ml_dtypes
import numpy as np

import concourse.bacc as bacc
import concourse.bass as bass
import concourse.bass_isa as bass_isa
import concourse.mybir as mybir
import concourse.tile as tile
from concourse.bass_utils import run_bass_kernel_spmd

F32 = mybir.dt.float32
BF16 = mybir.dt.bfloat16
FP8 = mybir.dt.float8e4
AF = mybir.ActivationFunctionType
ALU = mybir.AluOpType
DR = mybir.MatmulPerfMode.DoubleRow

N_CORES = 8
N = 8192
D = 1024
TEMPERATURE = 0.07


def build_nc(n=N, d=D, n_cores=N_CORES, no_collective=False, stop_after=None):
    """Build the SPMD Bass program (same program on every core)."""
    cexp = float(1.0 / TEMPERATURE)          # stabilizer: max possible logit
    rows = n // n_cores                      # image rows per core (1024)
    P = 128
    rp = rows // P                           # img row-tiles per core (8)
    kt = d // P                              # contraction sub-tiles (8)
    n_ch = n // P                            # txt column chunks of 128 (64)

    nc = bacc.Bacc("TRN2", target_bir_lowering=False, debug=False,
                   num_devices=n_cores)
    img = nc.dram_tensor("img", [rows, d], F32, kind="ExternalInput").ap()
    txt_t8 = nc.dram_tensor("txt_t8", [d, n], FP8, kind="ExternalInput").ap()
    txt_own = nc.dram_tensor("txt_own", [rows, d], F32, kind="ExternalInput").ap()
    ones = nc.dram_tensor("ones", [P, P], F32, kind="ExternalInput").ap()
    ones_b = nc.dram_tensor("ones_b", [P, P], BF16, kind="ExternalInput").ap()
    ident = nc.dram_tensor("ident", [P, P], BF16, kind="ExternalInput").ap()
    out = nc.dram_tensor("out", [1, 1], F32, kind="ExternalOutput").ap()

    with tile.TileContext(nc) as tc:
        _body(tc, img, txt_t8, txt_own, ones, ones_b, ident, out,
              n=n, d=d, rows=rows, P=P, rp=rp, kt=kt, n_ch=n_ch,
              cexp=cexp, n_cores=n_cores, no_collective=no_collective,
              stop_after=stop_after)
    nc.compile()
    return nc


def _body(tc, img, txt_t8, txt_own, ones, ones_b, ident, out, *, n, d, rows,
          P, rp, kt, n_ch, cexp, n_cores, no_collective, stop_after=None):
    nc = tc.nc
    ln_inv_t = float(np.log(1.0 / TEMPERATURE))
    with ExitStack() as ctx:
        persist = ctx.enter_context(tc.tile_pool(name="persist", bufs=1))
        sqp = ctx.enter_context(tc.tile_pool(name="sqp", bufs=2))
        nrm = ctx.enter_context(tc.tile_pool(name="nrm", bufs=2))
        exp_p = ctx.enter_context(tc.tile_pool(name="exp_p", bufs=4))
        v1 = ctx.enter_context(tc.tile_pool(name="v1", bufs=4))
        ex_ps = ctx.enter_context(tc.tile_pool(name="ex_ps", bufs=3, space="PSUM"))
        tp_ps = ctx.enter_context(tc.tile_pool(name="tp_ps", bufs=2, space="PSUM"))
        dram = ctx.enter_context(tc.tile_pool(name="dram", bufs=1, space="DRAM"))

        txtT8 = persist.tile([P, kt, n], FP8, tag="txtT8")      # [d-part, k, j]
        imgT8 = persist.tile([P, kt, rows], FP8, tag="imgT8")   # [d-part, k, i]
        racc = persist.tile([P, rows], BF16, tag="racc")        # rowsum partial
        csacc = persist.tile([P, n_ch], F32, tag="csacc")       # colsum partial
        rcpT = persist.tile([P, n_ch], F32, tag="rcpT")         # 1/(T*|txt_j|)
        vecs = persist.tile([P, 40], F32, tag="vecs")
        ones_sb = persist.tile([P, P], F32, tag="ones")
        ones_bsb = persist.tile([P, P], BF16, tag="ones_bsb")
        ident_sb = persist.tile([P, P], BF16, tag="ident")
        cs_sb = persist.tile([P, n_ch], F32, tag="cs_sb")
        ln_cs = persist.tile([P, n_ch], BF16, tag="ln_cs")
        ebias = persist.tile([P, 1], F32, tag="ebias")
        lnb = persist.tile([P, 1], F32, tag="lnb")
        sc = persist.tile([P, 8], F32, tag="sc")

        cbuf_n = dram.tile([1, rows], F32, tag="cbuf_n")
        cbuf_n_out = dram.tile([1, n], F32, tag="cbuf_n_out", addr_space="Shared")
        cbuf = dram.tile([1, n + 64], F32, tag="cbuf")
        cbuf_out = dram.tile([1, n + 64], F32, tag="cbuf_out", addr_space="Shared")

        # vecs column map
        DG = 0           # diag partials (dot * r_img * rcp_txt/T)
        RQ = 8           # per-row-tile txt rcp/T (1/(T*|txt_i|))
        TS = 16          # txt_own ssq, later raw diag dots
        IS = 24          # img ssq
        RI = 32          # img rsqrt

        nc.sync.dma_start(ones_sb[:], ones[:])
        nc.sync.dma_start(ones_bsb[:], ones_b[:])
        nc.sync.dma_start(ident_sb[:], ident[:])
        nc.gpsimd.memset(ebias[:], float(-cexp))
        nc.gpsimd.memset(lnb[:], ln_inv_t)

        # full text matrix load (fp8, pre-transposed on host)
        for k in range(kt):
            nc.sync.dma_start(txtT8[:, k, :], txt_t8[k * P:(k + 1) * P, :])

        # --- Phase A1: txt_own norms (own text cols) -> rcp scales -----------
        to_hold = persist.tile([P, rp, d], F32, tag="to_hold")
        for t in range(rp):
            nc.sync.dma_start(to_hold[:, t, :], txt_own[t * P:(t + 1) * P, :])
        for t in range(rp):
            sq = sqp.tile([P, d], BF16, tag="sq")
            nc.scalar.activation(sq[:], to_hold[:, t, :], AF.Square,
                                 accum_out=vecs[:, TS + t:TS + t + 1])
        lt = v1.tile([P, 8], F32, tag="v1")
        nc.scalar.activation(lt[:, 0:rp], vecs[:, TS:TS + rp], AF.Ln)
        # 1/(T*||txt_own_i||) = exp(-0.5*ln(ssq) + ln(1/T))
        nc.scalar.activation(vecs[:, RQ:RQ + rp], lt[:, 0:rp],
                             AF.Exp, scale=-0.5, bias=lnb[:, 0:1])
        # ship own rcp scales; AllGather to all cores
        nc.sync.dma_start(
            cbuf_n[0:1, :].rearrange("a (x p) -> (a p) x", p=P),
            vecs[:, RQ:RQ + rp])
        if no_collective:
            # debug: replicate local scales into every chunk slot (wrong
            # values off-shard, but exercises the full pipeline)
            for r in range(n_cores):
                nc.sync.dma_start(
                    rcpT[:, r * rp:(r + 1) * rp],
                    cbuf_n[0:1, :].rearrange("a (x p) -> (a p) x", p=P))
        else:
            nc.gpsimd.collective_compute(
                "AllGather", ALU.bypass,
                replica_groups=[list(range(n_cores))],
                ins=[cbuf_n[:].opt()], outs=[cbuf_n_out[:].opt()])
            nc.sync.dma_start(
                rcpT[:],
                cbuf_n_out[0:1, :].rearrange("a (x p) -> (a p) x", p=P))

        # --- Phase A2: img prep (normalize, diag dots, transpose to fp8) -----
        img_hold = persist.tile([P, rp, d], F32, tag="img_hold")
        for t in range(rp):
            nc.sync.dma_start(img_hold[:, t, :], img[t * P:(t + 1) * P, :])
        for t in range(rp):
            sq = sqp.tile([P, d], BF16, tag="sq")
            nc.scalar.activation(sq[:], img_hold[:, t, :], AF.Square,
                                 accum_out=vecs[:, IS + t:IS + t + 1])
        li = v1.tile([P, 8], F32, tag="v1")
        nc.scalar.activation(li[:, 0:rp], vecs[:, IS:IS + rp], AF.Ln)
        nc.scalar.activation(vecs[:, RI:RI + rp], li[:, 0:rp],
                             AF.Exp, scale=-0.5)
        for t in range(rp):
            # diag partial: dot(img_i, txt_own_i) * r_img * (rcp_txt/T)
            v = v1.tile([P, 8], F32, tag="v1")
            dsc = sqp.tile([P, d], BF16, tag="sq")
            nc.vector.tensor_tensor(dsc[:], img_hold[:, t, :],
                                    to_hold[:, t, :], ALU.mult)
            nc.vector.tensor_reduce(v[:, 0:1], dsc[:],
                                    axis=mybir.AxisListType.X, op=ALU.add)
            nc.vector.tensor_tensor(v[:, 1:2], vecs[:, RI + t:RI + t + 1],
                                    vecs[:, RQ + t:RQ + t + 1], ALU.mult)
            nc.vector.tensor_tensor(vecs[:, DG + t:DG + t + 1], v[:, 0:1],
                                    v[:, 1:2], ALU.mult)
            # normalize img rows -> bf16, transpose k-tiles to fp8 imgT8
            imgn_b = nrm.tile([P, d], BF16, tag="imgn")
            nc.vector.tensor_scalar_mul(imgn_b[:], img_hold[:, t, :],
                                        vecs[:, RI + t:RI + t + 1])
            for k in range(kt):
                tp = tp_ps.tile([P, P], BF16, tag="tp")
                nc.tensor.transpose(tp[:], imgn_b[:, k * P:(k + 1) * P],
                                    ident_sb[:])
                nc.vector.tensor_copy(imgT8[:, k, t * P:(t + 1) * P], tp[:])

        if stop_after == "A":
            nc.sync.dma_start(out[0:1, 0:1], vecs[0:1, DG:DG + 1])
            return

        # --- Phase C: main fp8 DoubleRow matmul + exp + reductions -----------
        HB = rows // 512                     # img halves per chunk (2)
        for c in range(n_ch):
            mm = ex_ps.tile([P, rows], F32, tag="ex")
            for t in range(kt // 2):
                for h in range(HB):
                    nc.tensor.matmul(
                        mm[:, h * 512:(h + 1) * 512],
                        txtT8[:, 2 * t:2 * t + 2, c * P:(c + 1) * P],
                        imgT8[:, 2 * t:2 * t + 2, h * 512:(h + 1) * 512],
                        start=(t == 0), stop=(t == kt // 2 - 1),
                        perf_mode=DR)
            ex = exp_p.tile([P, rows], BF16, tag="exp")
            nc.scalar.activation(ex[:], mm[:], AF.Exp,
                                 bias=ebias[:, 0:1], scale=rcpT[:, c:c + 1],
                                 accum_out=csacc[:, c:c + 1])
            if c == 0:
                nc.vector.tensor_copy(racc[:], ex[:])
            else:
                nc.vector.tensor_tensor(racc[:], racc[:], ex[:], ALU.add)

        if stop_after == "C":
            nc.sync.dma_start(out[0:1, 0:1], csacc[0:1, 0:1])
            return

        # --- Phase D: local scalars ------------------------------------------
        # R_m = sum_i ln(rowsum_i): partition-reduce racc via ones-matmul
        for h in range(HB):
            rs = ex_ps.tile([P, rows], F32, tag="ex")
            nc.tensor.matmul(rs[0:1, 0:512], ones_bsb[:, 0:1],
                             racc[:, h * 512:(h + 1) * 512],
                             start=True, stop=True)
            lnr = v1.tile([P, 512], BF16, tag="lnr")
            nc.scalar.activation(lnr[0:1, :], rs[0:1, 0:512], AF.Ln,
                                 accum_out=sc[0:1, 2 + h:3 + h])
        nc.vector.tensor_tensor(sc[0:1, 0:1], sc[0:1, 2:3], sc[0:1, 3:4],
                                ALU.add)                         # R_m
        # Draw'_m
        dg1 = v1.tile([P, 8], F32, tag="v1")
        nc.vector.tensor_reduce(dg1[:, 0:1], vecs[:, DG:DG + rp],
                                axis=mybir.AxisListType.X, op=ALU.add)
        dr = ex_ps.tile([P, rows], F32, tag="ex")
        nc.tensor.matmul(dr[0:1, 0:1], ones_sb[:, 0:1], dg1[:, 0:1],
                         start=True, stop=True)
        nc.vector.tensor_copy(sc[0:1, 1:2], dr[0:1, 0:1])        # Draw'_m

        # ship partials: [colsums(8192), R_m, Draw'_m]
        nc.sync.dma_start(
            cbuf[0:1, 0:n].rearrange("a (x p) -> (a p) x", p=P), csacc[:])
        nc.sync.dma_start(cbuf[0:1, n:n + 2], sc[0:1, 0:2])

        if stop_after == "D":
            nc.sync.dma_start(out[0:1, 0:1], sc[0:1, 0:1])
            return

        # --- Phase E: AllReduce + finish -------------------------------------
        if no_collective:
            nc.sync.dma_start(cbuf_out[:], cbuf[:])
        else:
            nc.gpsimd.collective_compute(
                "AllReduce", ALU.add,
                replica_groups=[list(range(n_cores))],
                ins=[cbuf[:].opt()], outs=[cbuf_out[:].opt()])

        nc.sync.dma_start(
            cs_sb[:], cbuf_out[0:1, 0:n].rearrange("a (x p) -> (a p) x", p=P))
        lacc = v1.tile([P, 8], F32, tag="v1")
        nc.scalar.activation(ln_cs[:], cs_sb[:], AF.Ln,
                             accum_out=lacc[:, 0:1])
        lps = ex_ps.tile([P, rows], F32, tag="ex")
        nc.tensor.matmul(lps[0:1, 0:1], ones_sb[:, 0:1], lacc[:, 0:1],
                         start=True, stop=True)                  # L
        rd = v1.tile([P, 8], F32, tag="v1")
        nc.sync.dma_start(rd[0:1, 0:2], cbuf_out[0:1, n:n + 2])

        # loss = cexp + (R + L - 2 * Draw') / (2N)
        fin = v1.tile([P, 8], F32, tag="v1")
        nc.vector.tensor_tensor(fin[0:1, 0:1], rd[0:1, 0:1], lps[0:1, 0:1],
                                ALU.add)                         # R + L
        nc.vector.tensor_scalar_mul(fin[0:1, 1:2], rd[0:1, 1:2], -2.0)
        nc.vector.tensor_tensor(fin[0:1, 2:3], fin[0:1, 0:1], fin[0:1, 1:2],
                                ALU.add)
        nc.scalar.activation(fin[0:1, 3:4], fin[0:1, 2:3], AF.Copy,
                             bias=float(cexp), scale=float(1.0 / (2 * n)))
        nc.sync.dma_start(out[0:1, 0:1], fin[0:1, 3:4])


def make_in_maps(image_features, text_features, n=N, d=D, n_cores=N_CORES):
    image_features = np.asarray(image_features, dtype=np.float32)
    text_features = np.asarray(text_features, dtype=np.float32)
    rows = n // n_cores
    txt_t8 = np.ascontiguousarray(text_features.T).astype(ml_dtypes.float8_e4m3)
    ones = np.ones((128, 128), dtype=np.float32)
    ones_b = np.ones((128, 128), dtype=ml_dtypes.bfloat16)
    ident = np.eye(128, dtype=np.float32).astype(ml_dtypes.bfloat16)
    return [
        {
            "img": image_features[m * rows:(m + 1) * rows],
            "txt_t8": txt_t8,
            "txt_own": text_features[m * rows:(m + 1) * rows],
            "ones": ones,
            "ones_b": ones_b,
            "ident": ident,
        }
        for m in range(n_cores)
    ]


_CACHE = {}
_LOCK = threading.Lock()


def _get_nc():
    with _LOCK:
        if "nc" not in _CACHE:
            _CACHE["nc"] = build_nc()
        return _CACHE["nc"]


def kernel(image_features, text_features):
    image_features = np.asarray(image_features, dtype=np.float32)
    text_features = np.asarray(text_features, dtype=np.float32)
    assert image_features.shape == (N, D) and text_features.shape == (N, D)
    nc = _get_nc()
    in_maps = make_in_maps(image_features, text_features)
    res = run_bass_kernel_spmd(nc, in_maps, list(range(N_CORES)))
    val = np.float32(res.results[0]["out"][0, 0])
    return np.array(val, dtype=np.float32)


# revision 26
# speedup vs baseline: 1.8475x; 1.1041x over previous
"""Trainium2 Bass kernel for CLIP-style symmetric contrastive loss.

Problem: image_features [8192, 1024] f32, text_features [8192, 1024] f32.
  loss = 0.5 * (CE(logits, diag) + CE(logits.T, diag)),
  logits = cosine_similarity(img, txt) / 0.07.

Distribution: shard image rows across 8 NeuronCores. Each core computes the
transposed slab S^T = txt_raw^T @ img_n  ([8192 txt cols, 1024 img rows]) in
64 chunks of 128 txt columns, using fp8 DoubleRow matmuls (txt raw fp8 as the
stationary operand, normalized img fp8 as the moving operand). The per-txt-col
normalization 1/(T*||txt_j||) folds into the exp activation's per-partition
scale, so the text matrix is never normalized explicitly. Activation
accum_out yields column sums for free; row sums accumulate on the Vector
engine in bf16 and reduce via a ones-matmul.

Per-core text-column norms come from the txt_own diag pass (sum-of-squares of
this core's text rows == this core's 1024 text columns); an early 4 KB
AllGather distributes the reciprocal scales to every core. A single [8194]
AllReduce at the end combines column sums + per-core scalar partials.

Math (C = 1/T upper-bounds every logit, so exp(S - C) <= 1 is stable):
  loss = C + (R + L - 2 * Draw') / (2N)
    R     = sum_i log sum_j exp(S_ij - C)
    L     = sum_j log sum_i exp(S_ij - C)
    Draw' = sum_i cos(img_i, txt_i) / T
"""
import threading
from contextlib import ExitStack

import ml_dtypes
import numpy as np

import concourse.bacc as bacc
import concourse.bass as bass
import concourse.bass_isa as bass_isa
import concourse.mybir as mybir
import concourse.tile as tile
from concourse.bass_utils import run_bass_kernel_spmd

F32 = mybir.dt.float32
BF16 = mybir.dt.bfloat16
FP8 = mybir.dt.float8e4
AF = mybir.ActivationFunctionType
ALU = mybir.AluOpType
DR = mybir.MatmulPerfMode.DoubleRow

N_CORES = 8
N = 8192
D = 1024
TEMPERATURE = 0.07


def build_nc(n=N, d=D, n_cores=N_CORES, no_collective=False, stop_after=None):
    """Build the SPMD Bass program (same program on every core)."""
    cexp = float(1.0 / TEMPERATURE)          # stabilizer: max possible logit
    rows = n // n_cores                      # image rows per core (1024)
    P = 128
    rp = rows // P                           # img row-tiles per core (8)
    kt = d // P                              # contraction sub-tiles (8)
    n_ch = n // P                            # txt column chunks of 128 (64)

    nc = bacc.Bacc("TRN2", target_bir_lowering=False, debug=False,
                   num_devices=n_cores)
    img = nc.dram_tensor("img", [rows, d], F32, kind="ExternalInput").ap()
    txt_t8 = nc.dram_tensor("txt_t8", [d, n], FP8, kind="ExternalInput").ap()
    txt_own = nc.dram_tensor("txt_own", [rows, d], F32, kind="ExternalInput").ap()
    ones = nc.dram_tensor("ones", [P, P], F32, kind="ExternalInput").ap()
    ones_b = nc.dram_tensor("ones_b", [P, P], BF16, kind="ExternalInput").ap()
    ident = nc.dram_tensor("ident", [P, P], BF16, kind="ExternalInput").ap()
    out = nc.dram_tensor("out", [1, 1], F32, kind="ExternalOutput").ap()

    with tile.TileContext(nc) as tc:
        _body(tc, img, txt_t8, txt_own, ones, ones_b, ident, out,
              n=n, d=d, rows=rows, P=P, rp=rp, kt=kt, n_ch=n_ch,
              cexp=cexp, n_cores=n_cores, no_collective=no_collective,
              stop_after=stop_after)
    nc.compile()
    return nc


def _body(tc, img, txt_t8, txt_own, ones, ones_b, ident, out, *, n, d, rows,
          P, rp, kt, n_ch, cexp, n_cores, no_collective, stop_after=None):
    nc = tc.nc
    ln_inv_t = float(np.log(1.0 / TEMPERATURE))
    with ExitStack() as ctx:
        persist = ctx.enter_context(tc.tile_pool(name="persist", bufs=1))
        sqp = ctx.enter_context(tc.tile_pool(name="sqp", bufs=2))
        nrm = ctx.enter_context(tc.tile_pool(name="nrm", bufs=2))
        exp_p = ctx.enter_context(tc.tile_pool(name="exp_p", bufs=4))
        v1 = ctx.enter_context(tc.tile_pool(name="v1", bufs=4))
        ex_ps = ctx.enter_context(tc.tile_pool(name="ex_ps", bufs=3, space="PSUM"))
        tp_ps = ctx.enter_context(tc.tile_pool(name="tp_ps", bufs=2, space="PSUM"))
        dram = ctx.enter_context(tc.tile_pool(name="dram", bufs=1, space="DRAM"))

        txtT8 = persist.tile([P, kt, n], FP8, tag="txtT8")      # [d-part, k, j]
        imgT8 = persist.tile([P, kt, rows], FP8, tag="imgT8")   # [d-part, k, i]
        racc = persist.tile([P, rows], BF16, tag="racc")        # rowsum partial
        csacc = persist.tile([P, n_ch], F32, tag="csacc")       # colsum partial
        rcpT = persist.tile([P, n_ch], F32, tag="rcpT")         # 1/(T*|txt_j|)
        vecs = persist.tile([P, 40], F32, tag="vecs")
        ones_sb = persist.tile([P, P], F32, tag="ones")
        ones_bsb = persist.tile([P, P], BF16, tag="ones_bsb")
        ident_sb = persist.tile([P, P], BF16, tag="ident")
        cs_sb = persist.tile([P, n_ch], F32, tag="cs_sb")
        ln_cs = persist.tile([P, n_ch], BF16, tag="ln_cs")
        ebias = persist.tile([P, 1], F32, tag="ebias")
        lnb = persist.tile([P, 1], F32, tag="lnb")
        sc = persist.tile([P, 8], F32, tag="sc")

        cbuf_n = dram.tile([1, rows], F32, tag="cbuf_n")
        cbuf_n_out = dram.tile([1, n], F32, tag="cbuf_n_out", addr_space="Shared")
        half = n // 2
        cbuf1 = dram.tile([1, half], F32, tag="cbuf1")
        cbuf1_out = dram.tile([1, half], F32, tag="cbuf1_out", addr_space="Shared")
        cbuf2 = dram.tile([1, half + 8], F32, tag="cbuf2")
        cbuf2_out = dram.tile([1, half + 8], F32, tag="cbuf2_out",
                              addr_space="Shared")

        # vecs column map
        DG = 0           # diag partials (dot * r_img * rcp_txt/T)
        RQ = 8           # per-row-tile txt rcp/T (1/(T*|txt_i|))
        TS = 16          # txt_own ssq, later raw diag dots
        IS = 24          # img ssq
        RI = 32          # img rsqrt

        nc.sync.dma_start(ones_sb[:], ones[:])
        nc.sync.dma_start(ones_bsb[:], ones_b[:])
        nc.sync.dma_start(ident_sb[:], ident[:])
        nc.gpsimd.memset(ebias[:], float(-cexp))
        nc.gpsimd.memset(lnb[:], ln_inv_t)

        # --- Phase A1: txt_own norms (own text cols) -> rcp scales -----------
        # DMA priority: txt_own (feeds the early AllGather), then img, then
        # the big text matrix (only needed once matmuls start).
        to_hold = persist.tile([P, rp, d], F32, tag="to_hold")
        img_hold = persist.tile([P, rp, d], F32, tag="img_hold")
        for t in range(rp):
            nc.sync.dma_start(to_hold[:, t, :], txt_own[t * P:(t + 1) * P, :])
        for t in range(rp):
            nc.sync.dma_start(img_hold[:, t, :], img[t * P:(t + 1) * P, :])
        for k in range(kt):
            nc.sync.dma_start(txtT8[:, k, :], txt_t8[k * P:(k + 1) * P, :])
        for t in range(rp):
            sq = sqp.tile([P, d], BF16, tag="sq")
            nc.scalar.activation(sq[:], to_hold[:, t, :], AF.Square,
                                 accum_out=vecs[:, TS + t:TS + t + 1])
        lt = v1.tile([P, 8], F32, tag="v1")
        nc.scalar.activation(lt[:, 0:rp], vecs[:, TS:TS + rp], AF.Ln)
        # 1/(T*||txt_own_i||) = exp(-0.5*ln(ssq) + ln(1/T))
        nc.scalar.activation(vecs[:, RQ:RQ + rp], lt[:, 0:rp],
                             AF.Exp, scale=-0.5, bias=lnb[:, 0:1])
        # ship own rcp scales; AllGather to all cores
        nc.sync.dma_start(
            cbuf_n[0:1, :].rearrange("a (x p) -> (a p) x", p=P),
            vecs[:, RQ:RQ + rp])
        if no_collective:
            # debug: replicate local scales into every chunk slot (wrong
            # values off-shard, but exercises the full pipeline)
            for r in range(n_cores):
                nc.sync.dma_start(
                    rcpT[:, r * rp:(r + 1) * rp],
                    cbuf_n[0:1, :].rearrange("a (x p) -> (a p) x", p=P))
        else:
            nc.gpsimd.collective_compute(
                "AllGather", ALU.bypass,
                replica_groups=[list(range(n_cores))],
                ins=[cbuf_n[:].opt()], outs=[cbuf_n_out[:].opt()])
            nc.sync.dma_start(
                rcpT[:],
                cbuf_n_out[0:1, :].rearrange("a (x p) -> (a p) x", p=P))

        # --- Phase A2: img prep (normalize, transpose to fp8, diag dots) -----
        for t in range(rp):
            sq = sqp.tile([P, d], BF16, tag="sq")
            nc.scalar.activation(sq[:], img_hold[:, t, :], AF.Square,
                                 accum_out=vecs[:, IS + t:IS + t + 1])
        li = v1.tile([P, 8], F32, tag="v1")
        nc.scalar.activation(li[:, 0:rp], vecs[:, IS:IS + rp], AF.Ln)
        nc.scalar.activation(vecs[:, RI:RI + rp], li[:, 0:rp],
                             AF.Exp, scale=-0.5)
        img_n = persist.tile([P, rp, d], BF16, tag="img_n")
        for t in range(rp):
            nc.vector.tensor_scalar_mul(img_n[:, t, :], img_hold[:, t, :],
                                        vecs[:, RI + t:RI + t + 1])
        # k-outer so the first k-planes of imgT8 complete early and matmuls
        # can start while later planes still transpose
        for k in range(kt):
            for t in range(rp):
                tp = tp_ps.tile([P, P], BF16, tag="tp")
                nc.tensor.transpose(tp[:], img_n[:, t, k * P:(k + 1) * P],
                                    ident_sb[:])
                nc.vector.tensor_copy(imgT8[:, k, t * P:(t + 1) * P], tp[:])
        # diag partials are only needed at phase D; keep off critical path
        for t in range(rp):
            v = v1.tile([P, 8], F32, tag="v1")
            dsc = sqp.tile([P, d], BF16, tag="sq")
            nc.vector.tensor_tensor(dsc[:], img_hold[:, t, :],
                                    to_hold[:, t, :], ALU.mult)
            nc.vector.tensor_reduce(v[:, 0:1], dsc[:],
                                    axis=mybir.AxisListType.X, op=ALU.add)
            nc.vector.tensor_tensor(v[:, 1:2], vecs[:, RI + t:RI + t + 1],
                                    vecs[:, RQ + t:RQ + t + 1], ALU.mult)
            nc.vector.tensor_tensor(vecs[:, DG + t:DG + t + 1], v[:, 0:1],
                                    v[:, 1:2], ALU.mult)

        if stop_after == "A":
            nc.sync.dma_start(out[0:1, 0:1], vecs[0:1, DG:DG + 1])
            return

        # --- Phase C: main fp8 DoubleRow matmul + exp + reductions -----------
        HB = rows // 512                     # img halves per chunk (2)
        hch = n_ch // 2
        for c in range(n_ch):
            mm = ex_ps.tile([P, rows], F32, tag="ex")
            for t in range(kt // 2):
                for h in range(HB):
                    nc.tensor.matmul(
                        mm[:, h * 512:(h + 1) * 512],
                        txtT8[:, 2 * t:2 * t + 2, c * P:(c + 1) * P],
                        imgT8[:, 2 * t:2 * t + 2, h * 512:(h + 1) * 512],
                        start=(t == 0), stop=(t == kt // 2 - 1),
                        perf_mode=DR)
            ex = exp_p.tile([P, rows], BF16, tag="exp")
            nc.scalar.activation(ex[:], mm[:], AF.Exp,
                                 bias=ebias[:, 0:1], scale=rcpT[:, c:c + 1],
                                 accum_out=csacc[:, c:c + 1])
            if c == 0:
                nc.vector.tensor_copy(racc[:], ex[:])
            else:
                nc.vector.tensor_tensor(racc[:], racc[:], ex[:], ALU.add)
            if c == hch - 1:
                # first half of colsums is complete: overlap its AllReduce
                # with the second half of the GEMM
                nc.sync.dma_start(
                    cbuf1[0:1, :].rearrange("a (x p) -> (a p) x", p=P),
                    csacc[:, 0:hch])
                if no_collective:
                    nc.sync.dma_start(cbuf1_out[:], cbuf1[:])
                else:
                    nc.gpsimd.collective_compute(
                        "AllReduce", ALU.add,
                        replica_groups=[list(range(n_cores))],
                        ins=[cbuf1[:].opt()], outs=[cbuf1_out[:].opt()])

        if stop_after == "C":
            nc.sync.dma_start(out[0:1, 0:1], csacc[0:1, 0:1])
            return

        # --- Phase D: local scalars ------------------------------------------
        # R_m = sum_i ln(rowsum_i): partition-reduce racc via ones-matmul
        for h in range(HB):
            rs = ex_ps.tile([P, rows], F32, tag="ex")
            nc.tensor.matmul(rs[0:1, 0:512], ones_bsb[:, 0:1],
                             racc[:, h * 512:(h + 1) * 512],
                             start=True, stop=True)
            lnr = v1.tile([P, 512], BF16, tag="lnr")
            nc.scalar.activation(lnr[0:1, :], rs[0:1, 0:512], AF.Ln,
                                 accum_out=sc[0:1, 2 + h:3 + h])
        nc.vector.tensor_tensor(sc[0:1, 0:1], sc[0:1, 2:3], sc[0:1, 3:4],
                                ALU.add)                         # R_m
        # Draw'_m
        dg1 = v1.tile([P, 8], F32, tag="v1")
        nc.vector.tensor_reduce(dg1[:, 0:1], vecs[:, DG:DG + rp],
                                axis=mybir.AxisListType.X, op=ALU.add)
        dr = ex_ps.tile([P, rows], F32, tag="ex")
        nc.tensor.matmul(dr[0:1, 0:1], ones_sb[:, 0:1], dg1[:, 0:1],
                         start=True, stop=True)
        nc.vector.tensor_copy(sc[0:1, 1:2], dr[0:1, 0:1])        # Draw'_m

        # ship partials: [colsums second half (4096), R_m, Draw'_m]
        nc.sync.dma_start(
            cbuf2[0:1, 0:half].rearrange("a (x p) -> (a p) x", p=P),
            csacc[:, hch:n_ch])
        nc.sync.dma_start(cbuf2[0:1, half:half + 2], sc[0:1, 0:2])

        if stop_after == "D":
            nc.sync.dma_start(out[0:1, 0:1], sc[0:1, 0:1])
            return

        # --- Phase E: second AllReduce + finish ------------------------------
        if no_collective:
            nc.sync.dma_start(cbuf2_out[:], cbuf2[:])
        else:
            nc.gpsimd.collective_compute(
                "AllReduce", ALU.add,
                replica_groups=[list(range(n_cores))],
                ins=[cbuf2[:].opt()], outs=[cbuf2_out[:].opt()])

        # ln of global colsums (first half overlaps the GEMM already)
        nc.sync.dma_start(
            cs_sb[:, 0:hch],
            cbuf1_out[0:1, :].rearrange("a (x p) -> (a p) x", p=P))
        lacc = v1.tile([P, 8], F32, tag="v1")
        nc.scalar.activation(ln_cs[:, 0:hch], cs_sb[:, 0:hch], AF.Ln,
                             accum_out=lacc[:, 0:1])
        nc.sync.dma_start(
            cs_sb[:, hch:n_ch],
            cbuf2_out[0:1, 0:half].rearrange("a (x p) -> (a p) x", p=P))
        nc.scalar.activation(ln_cs[:, hch:n_ch], cs_sb[:, hch:n_ch], AF.Ln,
                             accum_out=lacc[:, 1:2])
        nc.vector.tensor_tensor(lacc[:, 2:3], lacc[:, 0:1], lacc[:, 1:2],
                                ALU.add)
        lps = ex_ps.tile([P, rows], F32, tag="ex")
        nc.tensor.matmul(lps[0:1, 0:1], ones_sb[:, 0:1], lacc[:, 2:3],
                         start=True, stop=True)                  # L
        rd = v1.tile([P, 8], F32, tag="v1")
        nc.sync.dma_start(rd[0:1, 0:2], cbuf2_out[0:1, half:half + 2])

        # loss = cexp + (R + L - 2 * Draw') / (2N)
        fin = v1.tile([P, 8], F32, tag="v1")
        nc.vector.tensor_tensor(fin[0:1, 0:1], rd[0:1, 0:1], lps[0:1, 0:1],
                                ALU.add)                         # R + L
        nc.vector.tensor_scalar_mul(fin[0:1, 1:2], rd[0:1, 1:2], -2.0)
        nc.vector.tensor_tensor(fin[0:1, 2:3], fin[0:1, 0:1], fin[0:1, 1:2],
                                ALU.add)
        nc.scalar.activation(fin[0:1, 3:4], fin[0:1, 2:3], AF.Copy,
                             bias=float(cexp), scale=float(1.0 / (2 * n)))
        nc.sync.dma_start(out[0:1, 0:1], fin[0:1, 3:4])


def make_in_maps(image_features, text_features, n=N, d=D, n_cores=N_CORES):
    image_features = np.asarray(image_features, dtype=np.float32)
    text_features = np.asarray(text_features, dtype=np.float32)
    rows = n // n_cores
    txt_t8 = np.ascontiguousarray(text_features.T).astype(ml_dtypes.float8_e4m3)
    ones = np.ones((128, 128), dtype=np.float32)
    ones_b = np.ones((128, 128), dtype=ml_dtypes.bfloat16)
    ident = np.eye(128, dtype=np.float32).astype(ml_dtypes.bfloat16)
    return [
        {
            "img": image_features[m * rows:(m + 1) * rows],
            "txt_t8": txt_t8,
            "txt_own": text_features[m * rows:(m + 1) * rows],
            "ones": ones,
            "ones_b": ones_b,
            "ident": ident,
        }
        for m in range(n_cores)
    ]


_CACHE = {}
_LOCK = threading.Lock()


def _get_nc():
    with _LOCK:
        if "nc" not in _CACHE:
            _CACHE["nc"] = build_nc()
        return _CACHE["nc"]


def kernel(image_features, text_features):
    image_features = np.asarray(image_features, dtype=np.float32)
    text_features = np.asarray(text_features, dtype=np.float32)
    assert image_features.shape == (N, D) and text_features.shape == (N, D)
    nc = _get_nc()
    in_maps = make_in_maps(image_features, text_features)
    res = run_bass_kernel_spmd(nc, in_maps, list(range(N_CORES)))
    val = np.float32(res.results[0]["out"][0, 0])
    return np.array(val, dtype=np.float32)


# revision 28
# speedup vs baseline: 1.9244x; 1.0416x over previous
"""Trainium2 Bass kernel for CLIP-style symmetric contrastive loss.

Problem: image_features [8192, 1024] f32, text_features [8192, 1024] f32.
  loss = 0.5 * (CE(logits, diag) + CE(logits.T, diag)),
  logits = cosine_similarity(img, txt) / 0.07.

Distribution: shard image rows across 8 NeuronCores. Each core computes the
transposed slab S^T = txt_raw^T @ img_n  ([8192 txt cols, 1024 img rows]) in
64 chunks of 128 txt columns, using fp8 DoubleRow matmuls (txt raw fp8 as the
stationary operand, normalized img fp8 as the moving operand). The per-txt-col
normalization 1/(T*||txt_j||) folds into the exp activation's per-partition
scale, so the text matrix is never normalized explicitly. Activation
accum_out yields column sums for free; row sums accumulate on the Vector
engine in bf16 and reduce via a ones-matmul.

Per-core text-column norms come from the txt_own diag pass (sum-of-squares of
this core's text rows == this core's 1024 text columns); an early 4 KB
AllGather distributes the reciprocal scales to every core. A single [8194]
AllReduce at the end combines column sums + per-core scalar partials.

Math (C = 1/T upper-bounds every logit, so exp(S - C) <= 1 is stable):
  loss = C + (R + L - 2 * Draw') / (2N)
    R     = sum_i log sum_j exp(S_ij - C)
    L     = sum_j log sum_i exp(S_ij - C)
    Draw' = sum_i cos(img_i, txt_i) / T
"""
import threading
from contextlib import ExitStack

import ml_dtypes
import numpy as np

import concourse.bacc as bacc
import concourse.bass as bass
import concourse.bass_isa as bass_isa
import concourse.mybir as mybir
import concourse.tile as tile
from concourse.bass_utils import run_bass_kernel_spmd

F32 = mybir.dt.float32
BF16 = mybir.dt.bfloat16
FP8 = mybir.dt.float8e4
AF = mybir.ActivationFunctionType
ALU = mybir.AluOpType
DR = mybir.MatmulPerfMode.DoubleRow

N_CORES = 8
N = 8192
D = 1024
TEMPERATURE = 0.07


def build_nc(n=N, d=D, n_cores=N_CORES, no_collective=False, stop_after=None):
    """Build the SPMD Bass program (same program on every core)."""
    cexp = float(1.0 / TEMPERATURE)          # stabilizer: max possible logit
    rows = n // n_cores                      # image rows per core (1024)
    P = 128
    rp = rows // P                           # img row-tiles per core (8)
    kt = d // P                              # contraction sub-tiles (8)
    n_ch = n // P                            # txt column chunks of 128 (64)

    nc = bacc.Bacc("TRN2", target_bir_lowering=False, debug=False,
                   num_devices=n_cores)
    img = nc.dram_tensor("img", [rows, d], F32, kind="ExternalInput").ap()
    txt_t8 = nc.dram_tensor("txt_t8", [d, n], FP8, kind="ExternalInput").ap()
    txt_own = nc.dram_tensor("txt_own", [rows, d], F32, kind="ExternalInput").ap()
    ones = nc.dram_tensor("ones", [P, P], F32, kind="ExternalInput").ap()
    ones_b = nc.dram_tensor("ones_b", [P, P], BF16, kind="ExternalInput").ap()
    ident = nc.dram_tensor("ident", [P, P], BF16, kind="ExternalInput").ap()
    out = nc.dram_tensor("out", [1, 1], F32, kind="ExternalOutput").ap()

    with tile.TileContext(nc) as tc:
        _body(tc, img, txt_t8, txt_own, ones, ones_b, ident, out,
              n=n, d=d, rows=rows, P=P, rp=rp, kt=kt, n_ch=n_ch,
              cexp=cexp, n_cores=n_cores, no_collective=no_collective,
              stop_after=stop_after)
    nc.compile()
    return nc


def _body(tc, img, txt_t8, txt_own, ones, ones_b, ident, out, *, n, d, rows,
          P, rp, kt, n_ch, cexp, n_cores, no_collective, stop_after=None):
    nc = tc.nc
    ln_inv_t = float(np.log(1.0 / TEMPERATURE))
    with ExitStack() as ctx:
        persist = ctx.enter_context(tc.tile_pool(name="persist", bufs=1))
        sqp = ctx.enter_context(tc.tile_pool(name="sqp", bufs=2))
        nrm = ctx.enter_context(tc.tile_pool(name="nrm", bufs=2))
        exp_p = ctx.enter_context(tc.tile_pool(name="exp_p", bufs=4))
        v1 = ctx.enter_context(tc.tile_pool(name="v1", bufs=4))
        ex_ps = ctx.enter_context(tc.tile_pool(name="ex_ps", bufs=3, space="PSUM"))
        tp_ps = ctx.enter_context(tc.tile_pool(name="tp_ps", bufs=2, space="PSUM"))
        dram = ctx.enter_context(tc.tile_pool(name="dram", bufs=1, space="DRAM"))

        txtT8 = persist.tile([P, kt, n], FP8, tag="txtT8")      # [d-part, k, j]
        imgT8 = persist.tile([P, kt, rows], FP8, tag="imgT8")   # [d-part, k, i]
        racc = persist.tile([P, rows], BF16, tag="racc")        # rowsum partial
        csacc = persist.tile([P, n_ch], F32, tag="csacc")       # colsum partial
        rcpT = persist.tile([P, n_ch], F32, tag="rcpT")         # 1/(T*|txt_j|)
        vecs = persist.tile([P, 40], F32, tag="vecs")
        ones_sb = persist.tile([P, P], F32, tag="ones")
        ones_bsb = persist.tile([P, P], BF16, tag="ones_bsb")
        ident_sb = persist.tile([P, P], BF16, tag="ident")
        cs_sb = persist.tile([P, n_ch], F32, tag="cs_sb")
        ln_cs = persist.tile([P, n_ch], BF16, tag="ln_cs")
        ebias = persist.tile([P, 1], F32, tag="ebias")
        lnb = persist.tile([P, 1], F32, tag="lnb")
        sc = persist.tile([P, 8], F32, tag="sc")

        bar = dram.tile([1, 8], F32, tag="bar")
        bar_out = dram.tile([1, 8], F32, tag="bar_out", addr_space="Shared")
        cbuf_n = dram.tile([1, rows], F32, tag="cbuf_n")
        cbuf_n_out = dram.tile([1, n], F32, tag="cbuf_n_out", addr_space="Shared")
        half = n // 2
        cbuf1 = dram.tile([1, half], F32, tag="cbuf1")
        cbuf1_out = dram.tile([1, half], F32, tag="cbuf1_out", addr_space="Shared")
        cbuf2 = dram.tile([1, half + 8], F32, tag="cbuf2")
        cbuf2_out = dram.tile([1, half + 8], F32, tag="cbuf2_out",
                              addr_space="Shared")

        # vecs column map
        DG = 0           # diag partials (dot * r_img * rcp_txt/T)
        RQ = 8           # per-row-tile txt rcp/T (1/(T*|txt_i|))
        TS = 16          # txt_own ssq, later raw diag dots
        IS = 24          # img ssq
        RI = 32          # img rsqrt

        # early sync barrier: absorbs core launch skew while phase A's DMAs
        # and compute run, so the real AllGather below starts promptly
        if not no_collective:
            nc.gpsimd.collective_compute(
                "AllReduce", ALU.add,
                replica_groups=[list(range(n_cores))],
                ins=[bar[:].opt()], outs=[bar_out[:].opt()])

        nc.sync.dma_start(ones_sb[:], ones[:])
        nc.sync.dma_start(ones_bsb[:], ones_b[:])
        nc.sync.dma_start(ident_sb[:], ident[:])
        nc.gpsimd.memset(ebias[:], float(-cexp))
        nc.gpsimd.memset(lnb[:], ln_inv_t)

        # --- Phase A1: txt_own norms (own text cols) -> rcp scales -----------
        # DMA priority: txt_own (feeds the early AllGather), then img, then
        # the big text matrix (only needed once matmuls start).
        to_hold = persist.tile([P, rp, d], F32, tag="to_hold")
        img_hold = persist.tile([P, rp, d], F32, tag="img_hold")
        for t in range(rp):
            nc.sync.dma_start(to_hold[:, t, :], txt_own[t * P:(t + 1) * P, :])
        for t in range(rp):
            nc.sync.dma_start(img_hold[:, t, :], img[t * P:(t + 1) * P, :])
        for k in range(kt):
            nc.sync.dma_start(txtT8[:, k, :], txt_t8[k * P:(k + 1) * P, :])
        for t in range(rp):
            sq = sqp.tile([P, d], BF16, tag="sq")
            nc.scalar.activation(sq[:], to_hold[:, t, :], AF.Square,
                                 accum_out=vecs[:, TS + t:TS + t + 1])
        lt = v1.tile([P, 8], F32, tag="v1")
        nc.scalar.activation(lt[:, 0:rp], vecs[:, TS:TS + rp], AF.Ln)
        # 1/(T*||txt_own_i||) = exp(-0.5*ln(ssq) + ln(1/T))
        nc.scalar.activation(vecs[:, RQ:RQ + rp], lt[:, 0:rp],
                             AF.Exp, scale=-0.5, bias=lnb[:, 0:1])
        # ship own rcp scales; AllGather to all cores
        nc.sync.dma_start(
            cbuf_n[0:1, :].rearrange("a (x p) -> (a p) x", p=P),
            vecs[:, RQ:RQ + rp])
        if no_collective:
            # debug: replicate local scales into every chunk slot (wrong
            # values off-shard, but exercises the full pipeline)
            for r in range(n_cores):
                nc.sync.dma_start(
                    rcpT[:, r * rp:(r + 1) * rp],
                    cbuf_n[0:1, :].rearrange("a (x p) -> (a p) x", p=P))
        else:
            nc.gpsimd.collective_compute(
                "AllGather", ALU.bypass,
                replica_groups=[list(range(n_cores))],
                ins=[cbuf_n[:].opt()], outs=[cbuf_n_out[:].opt()])
            nc.sync.dma_start(
                rcpT[:],
                cbuf_n_out[0:1, :].rearrange("a (x p) -> (a p) x", p=P))

        # --- Phase A2: img prep (normalize, transpose to fp8, diag dots) -----
        for t in range(rp):
            sq = sqp.tile([P, d], BF16, tag="sq")
            nc.scalar.activation(sq[:], img_hold[:, t, :], AF.Square,
                                 accum_out=vecs[:, IS + t:IS + t + 1])
        li = v1.tile([P, 8], F32, tag="v1")
        nc.scalar.activation(li[:, 0:rp], vecs[:, IS:IS + rp], AF.Ln)
        nc.scalar.activation(vecs[:, RI:RI + rp], li[:, 0:rp],
                             AF.Exp, scale=-0.5)
        img_n = persist.tile([P, rp, d], BF16, tag="img_n")
        for t in range(rp):
            nc.vector.tensor_scalar_mul(img_n[:, t, :], img_hold[:, t, :],
                                        vecs[:, RI + t:RI + t + 1])
        # k-outer so the first k-planes of imgT8 complete early and matmuls
        # can start while later planes still transpose
        for k in range(kt):
            for t in range(rp):
                tp = tp_ps.tile([P, P], BF16, tag="tp")
                nc.tensor.transpose(tp[:], img_n[:, t, k * P:(k + 1) * P],
                                    ident_sb[:])
                nc.vector.tensor_copy(imgT8[:, k, t * P:(t + 1) * P], tp[:])
        # diag partials are only needed at phase D; keep off critical path
        for t in range(rp):
            v = v1.tile([P, 8], F32, tag="v1")
            dsc = sqp.tile([P, d], BF16, tag="sq")
            nc.vector.tensor_tensor(dsc[:], img_hold[:, t, :],
                                    to_hold[:, t, :], ALU.mult)
            nc.vector.tensor_reduce(v[:, 0:1], dsc[:],
                                    axis=mybir.AxisListType.X, op=ALU.add)
            nc.vector.tensor_tensor(v[:, 1:2], vecs[:, RI + t:RI + t + 1],
                                    vecs[:, RQ + t:RQ + t + 1], ALU.mult)
            nc.vector.tensor_tensor(vecs[:, DG + t:DG + t + 1], v[:, 0:1],
                                    v[:, 1:2], ALU.mult)

        if stop_after == "A":
            nc.sync.dma_start(out[0:1, 0:1], vecs[0:1, DG:DG + 1])
            return

        # --- Phase C: main fp8 DoubleRow matmul + exp + reductions -----------
        HB = rows // 512                     # img halves per chunk (2)
        hch = n_ch // 2
        for c in range(n_ch):
            mm = ex_ps.tile([P, rows], F32, tag="ex")
            for t in range(kt // 2):
                for h in range(HB):
                    nc.tensor.matmul(
                        mm[:, h * 512:(h + 1) * 512],
                        txtT8[:, 2 * t:2 * t + 2, c * P:(c + 1) * P],
                        imgT8[:, 2 * t:2 * t + 2, h * 512:(h + 1) * 512],
                        start=(t == 0), stop=(t == kt // 2 - 1),
                        perf_mode=DR)
            ex = exp_p.tile([P, rows], BF16, tag="exp")
            nc.scalar.activation(ex[:], mm[:], AF.Exp,
                                 bias=ebias[:, 0:1], scale=rcpT[:, c:c + 1],
                                 accum_out=csacc[:, c:c + 1])
            if c == 0:
                nc.vector.tensor_copy(racc[:], ex[:])
            else:
                nc.vector.tensor_tensor(racc[:], racc[:], ex[:], ALU.add)
            if c == hch - 1:
                # first half of colsums is complete: overlap its AllReduce
                # with the second half of the GEMM
                nc.sync.dma_start(
                    cbuf1[0:1, :].rearrange("a (x p) -> (a p) x", p=P),
                    csacc[:, 0:hch])
                if no_collective:
                    nc.sync.dma_start(cbuf1_out[:], cbuf1[:])
                else:
                    nc.gpsimd.collective_compute(
                        "AllReduce", ALU.add,
                        replica_groups=[list(range(n_cores))],
                        ins=[cbuf1[:].opt()], outs=[cbuf1_out[:].opt()])

        if stop_after == "C":
            nc.sync.dma_start(out[0:1, 0:1], csacc[0:1, 0:1])
            return

        # --- Phase D: local scalars ------------------------------------------
        # R_m = sum_i ln(rowsum_i): partition-reduce racc via ones-matmul
        for h in range(HB):
            rs = ex_ps.tile([P, rows], F32, tag="ex")
            nc.tensor.matmul(rs[0:1, 0:512], ones_bsb[:, 0:1],
                             racc[:, h * 512:(h + 1) * 512],
                             start=True, stop=True)
            lnr = v1.tile([P, 512], BF16, tag="lnr")
            nc.scalar.activation(lnr[0:1, :], rs[0:1, 0:512], AF.Ln,
                                 accum_out=sc[0:1, 2 + h:3 + h])
        nc.vector.tensor_tensor(sc[0:1, 0:1], sc[0:1, 2:3], sc[0:1, 3:4],
                                ALU.add)                         # R_m
        # Draw'_m
        dg1 = v1.tile([P, 8], F32, tag="v1")
        nc.vector.tensor_reduce(dg1[:, 0:1], vecs[:, DG:DG + rp],
                                axis=mybir.AxisListType.X, op=ALU.add)
        dr = ex_ps.tile([P, rows], F32, tag="ex")
        nc.tensor.matmul(dr[0:1, 0:1], ones_sb[:, 0:1], dg1[:, 0:1],
                         start=True, stop=True)
        nc.vector.tensor_copy(sc[0:1, 1:2], dr[0:1, 0:1])        # Draw'_m

        # ship partials: [colsums second half (4096), R_m, Draw'_m]
        nc.sync.dma_start(
            cbuf2[0:1, 0:half].rearrange("a (x p) -> (a p) x", p=P),
            csacc[:, hch:n_ch])
        nc.sync.dma_start(cbuf2[0:1, half:half + 2], sc[0:1, 0:2])

        if stop_after == "D":
            nc.sync.dma_start(out[0:1, 0:1], sc[0:1, 0:1])
            return

        # --- Phase E: second AllReduce + finish ------------------------------
        if no_collective:
            nc.sync.dma_start(cbuf2_out[:], cbuf2[:])
        else:
            nc.gpsimd.collective_compute(
                "AllReduce", ALU.add,
                replica_groups=[list(range(n_cores))],
                ins=[cbuf2[:].opt()], outs=[cbuf2_out[:].opt()])

        # ln of global colsums (first half overlaps the GEMM already)
        nc.sync.dma_start(
            cs_sb[:, 0:hch],
            cbuf1_out[0:1, :].rearrange("a (x p) -> (a p) x", p=P))
        lacc = v1.tile([P, 8], F32, tag="v1")
        nc.scalar.activation(ln_cs[:, 0:hch], cs_sb[:, 0:hch], AF.Ln,
                             accum_out=lacc[:, 0:1])
        nc.sync.dma_start(
            cs_sb[:, hch:n_ch],
            cbuf2_out[0:1, 0:half].rearrange("a (x p) -> (a p) x", p=P))
        nc.scalar.activation(ln_cs[:, hch:n_ch], cs_sb[:, hch:n_ch], AF.Ln,
                             accum_out=lacc[:, 1:2])
        nc.vector.tensor_tensor(lacc[:, 2:3], lacc[:, 0:1], lacc[:, 1:2],
                                ALU.add)
        lps = ex_ps.tile([P, rows], F32, tag="ex")
        nc.tensor.matmul(lps[0:1, 0:1], ones_sb[:, 0:1], lacc[:, 2:3],
                         start=True, stop=True)                  # L
        rd = v1.tile([P, 8], F32, tag="v1")
        nc.sync.dma_start(rd[0:1, 0:2], cbuf2_out[0:1, half:half + 2])

        # loss = cexp + (R + L - 2 * Draw') / (2N)
        fin = v1.tile([P, 8], F32, tag="v1")
        nc.vector.tensor_tensor(fin[0:1, 0:1], rd[0:1, 0:1], lps[0:1, 0:1],
                                ALU.add)                         # R + L
        nc.vector.tensor_scalar_mul(fin[0:1, 1:2], rd[0:1, 1:2], -2.0)
        nc.vector.tensor_tensor(fin[0:1, 2:3], fin[0:1, 0:1], fin[0:1, 1:2],
                                ALU.add)
        nc.scalar.activation(fin[0:1, 3:4], fin[0:1, 2:3], AF.Copy,
                             bias=float(cexp), scale=float(1.0 / (2 * n)))
        nc.sync.dma_start(out[0:1, 0:1], fin[0:1, 3:4])


def make_in_maps(image_features, text_features, n=N, d=D, n_cores=N_CORES):
    image_features = np.asarray(image_features, dtype=np.float32)
    text_features = np.asarray(text_features, dtype=np.float32)
    rows = n // n_cores
    txt_t8 = np.ascontiguousarray(text_features.T).astype(ml_dtypes.float8_e4m3)
    ones = np.ones((128, 128), dtype=np.float32)
    ones_b = np.ones((128, 128), dtype=ml_dtypes.bfloat16)
    ident = np.eye(128, dtype=np.float32).astype(ml_dtypes.bfloat16)
    return [
        {
            "img": image_features[m * rows:(m + 1) * rows],
            "txt_t8": txt_t8,
            "txt_own": text_features[m * rows:(m + 1) * rows],
            "ones": ones,
            "ones_b": ones_b,
            "ident": ident,
        }
        for m in range(n_cores)
    ]


_CACHE = {}
_LOCK = threading.Lock()


def _get_nc():
    with _LOCK:
        if "nc" not in _CACHE:
            _CACHE["nc"] = build_nc()
        return _CACHE["nc"]


def kernel(image_features, text_features):
    image_features = np.asarray(image_features, dtype=np.float32)
    text_features = np.asarray(text_features, dtype=np.float32)
    assert image_features.shape == (N, D) and text_features.shape == (N, D)
    nc = _get_nc()
    in_maps = make_in_maps(image_features, text_features)
    res = run_bass_kernel_spmd(nc, in_maps, list(range(N_CORES)))
    val = np.float32(res.results[0]["out"][0, 0])
    return np.array(val, dtype=np.float32)


# revision 32
# speedup vs baseline: 2.0065x; 1.0427x over previous
"""Trainium2 Bass kernel for CLIP-style symmetric contrastive loss.

Problem: image_features [8192, 1024] f32, text_features [8192, 1024] f32.
  loss = 0.5 * (CE(logits, diag) + CE(logits.T, diag)),
  logits = cosine_similarity(img, txt) / 0.07.

Distribution: shard image rows across 8 NeuronCores. Each core computes the
transposed slab S^T = txt_raw^T @ img_n  ([8192 txt cols, 1024 img rows]) in
64 chunks of 128 txt columns, using fp8 DoubleRow matmuls (txt raw fp8 as the
stationary operand, normalized img fp8 as the moving operand). The per-txt-col
normalization 1/(T*||txt_j||) folds into the exp activation's per-partition
scale, so the text matrix is never normalized explicitly. Activation
accum_out yields column sums for free; row sums accumulate on the Vector
engine in bf16 and reduce via a ones-matmul.

All per-core inputs ship host-transposed (d on partitions), so norms and diag
dots reduce via ones-matmuls whose psum output is partition-broadcast -- no
PE transposes anywhere. Per-core text-col norms AllGather early (4 KB); the
colsum AllReduce is split in half so the first half overlaps the GEMM. A
leading dummy AllReduce soaks up core-launch skew.

Math (C = 1/T upper-bounds every logit, so exp(S - C) <= 1 is stable):
  loss = C + (R + L - 2 * Draw') / (2N)
    R     = sum_i log sum_j exp(S_ij - C)
    L     = sum_j log sum_i exp(S_ij - C)
    Draw' = sum_i cos(img_i, txt_i) / T
"""
import threading
from contextlib import ExitStack

import ml_dtypes
import numpy as np

import concourse.bacc as bacc
import concourse.bass as bass
import concourse.bass_isa as bass_isa
import concourse.mybir as mybir
import concourse.tile as tile
from concourse.bass_utils import run_bass_kernel_spmd

F32 = mybir.dt.float32
BF16 = mybir.dt.bfloat16
FP8 = mybir.dt.float8e4
AF = mybir.ActivationFunctionType
ALU = mybir.AluOpType
DR = mybir.MatmulPerfMode.DoubleRow

N_CORES = 8
N = 8192
D = 1024
TEMPERATURE = 0.07


def build_nc(n=N, d=D, n_cores=N_CORES, no_collective=False, stop_after=None):
    """Build the SPMD Bass program (same program on every core)."""
    cexp = float(1.0 / TEMPERATURE)          # stabilizer: max possible logit
    rows = n // n_cores                      # image rows per core (1024)
    P = 128
    kt = d // P                              # contraction sub-tiles (8)
    n_ch = n // P                            # txt column chunks of 128 (64)

    nc = bacc.Bacc("TRN2", target_bir_lowering=False, debug=False,
                   num_devices=n_cores)
    img_t = nc.dram_tensor("img_t", [d, rows], F32, kind="ExternalInput").ap()
    txt_t8 = nc.dram_tensor("txt_t8", [d, n], FP8, kind="ExternalInput").ap()
    to_t = nc.dram_tensor("to_t", [d, rows], F32, kind="ExternalInput").ap()
    ones = nc.dram_tensor("ones", [P, P], F32, kind="ExternalInput").ap()
    ones_b = nc.dram_tensor("ones_b", [P, P], BF16, kind="ExternalInput").ap()
    out = nc.dram_tensor("out", [1, 1], F32, kind="ExternalOutput").ap()

    with tile.TileContext(nc) as tc:
        _body(tc, img_t, txt_t8, to_t, ones, ones_b, out,
              n=n, d=d, rows=rows, P=P, kt=kt, n_ch=n_ch,
              cexp=cexp, n_cores=n_cores, no_collective=no_collective,
              stop_after=stop_after)
    nc.compile()
    return nc


def _body(tc, img_t, txt_t8, to_t, ones, ones_b, out, *, n, d, rows,
          P, kt, n_ch, cexp, n_cores, no_collective, stop_after=None):
    nc = tc.nc
    ln_inv_t = float(np.log(1.0 / TEMPERATURE))
    with ExitStack() as ctx:
        persist = ctx.enter_context(tc.tile_pool(name="persist", bufs=1))
        sqp = ctx.enter_context(tc.tile_pool(name="sqp", bufs=3))
        exp_p = ctx.enter_context(tc.tile_pool(name="exp_p", bufs=4))
        v1 = ctx.enter_context(tc.tile_pool(name="v1", bufs=4))
        ex_ps = ctx.enter_context(tc.tile_pool(name="ex_ps", bufs=4, space="PSUM"))
        dram = ctx.enter_context(tc.tile_pool(name="dram", bufs=1, space="DRAM"))

        txtT8 = persist.tile([P, kt, n], FP8, tag="txtT8")      # [d-part, k, j]
        imgT8 = persist.tile([P, kt, rows], FP8, tag="imgT8")   # [d-part, k, i]
        to_h = persist.tile([P, kt, rows], F32, tag="to_h")
        img_h = persist.tile([P, kt, rows], F32, tag="img_h")
        racc = persist.tile([P, rows], BF16, tag="racc")        # rowsum partial
        csacc = persist.tile([P, n_ch], F32, tag="csacc")       # colsum partial
        rcpT = persist.tile([P, n_ch], F32, tag="rcpT")         # 1/(T*|txt_j|)
        li_bc = persist.tile([P, rows], F32, tag="li_bc")
        rimg_bc = persist.tile([P, rows], F32, tag="rimg_bc")
        rcpt_row = persist.tile([1, rows], F32, tag="rcpt_row")
        lt_row = persist.tile([1, rows], F32, tag="lt_row")
        ones_sb = persist.tile([P, P], F32, tag="ones")
        ones_bsb = persist.tile([P, P], BF16, tag="ones_bsb")
        cs_sb = persist.tile([P, n_ch], F32, tag="cs_sb")
        ln_cs = persist.tile([P, n_ch], BF16, tag="ln_cs")
        ebias = persist.tile([P, 1], F32, tag="ebias")
        lnb = persist.tile([P, 1], F32, tag="lnb")
        sc = persist.tile([P, 8], F32, tag="sc")

        bar = dram.tile([1, 8], F32, tag="bar")
        bar_out = dram.tile([1, 8], F32, tag="bar_out", addr_space="Shared")
        cbuf_n = dram.tile([1, rows], F32, tag="cbuf_n")
        cbuf_n_out = dram.tile([1, n], F32, tag="cbuf_n_out", addr_space="Shared")
        half = n // 2
        cbuf1 = dram.tile([1, half], F32, tag="cbuf1")
        cbuf1_out = dram.tile([1, half], F32, tag="cbuf1_out", addr_space="Shared")
        cbuf2 = dram.tile([1, half + 8], F32, tag="cbuf2")
        cbuf2_out = dram.tile([1, half + 8], F32, tag="cbuf2_out",
                              addr_space="Shared")

        # early sync barrier: absorbs core launch skew while phase A's DMAs
        # and compute run, so the real AllGather below starts promptly
        if not no_collective:
            nc.gpsimd.collective_compute(
                "AllReduce", ALU.add,
                replica_groups=[list(range(n_cores))],
                ins=[bar[:].opt()], outs=[bar_out[:].opt()])

        nc.sync.dma_start(ones_sb[:], ones[:])
        nc.sync.dma_start(ones_bsb[:], ones_b[:])
        nc.gpsimd.memset(ebias[:], float(-cexp))
        nc.gpsimd.memset(lnb[:], ln_inv_t)

        # DMA priority: txt_own (feeds the early AllGather), then img, then
        # the big text matrix (only needed once matmuls start)
        for k in range(kt):
            nc.sync.dma_start(to_h[:, k, :], to_t[k * P:(k + 1) * P, :])
        for k in range(kt):
            nc.sync.dma_start(img_h[:, k, :], img_t[k * P:(k + 1) * P, :])
        for k in range(kt):
            nc.sync.dma_start(txtT8[:, k, :], txt_t8[k * P:(k + 1) * P, :])

        # --- Phase A1: txt norms (own text cols) -> AllGather rcp scales -----
        ssq_to = ex_ps.tile([P, rows], F32, tag="ex")
        for k in range(kt):
            sq = sqp.tile([P, rows], BF16, tag="sq")
            nc.vector.tensor_tensor(sq[:], to_h[:, k, :], to_h[:, k, :],
                                    ALU.mult)
            for h in range(rows // 512):
                nc.tensor.matmul(ssq_to[:, h * 512:(h + 1) * 512],
                                 ones_bsb[:], sq[:, h * 512:(h + 1) * 512],
                                 start=(k == 0), stop=(k == kt - 1))
        # 1/(T*||txt_j||) = exp(-0.5*ln(ssq) + ln(1/T)); psum rows are all
        # equal after the ones-matmul, row 0 suffices
        nc.scalar.activation(lt_row[0:1, :], ssq_to[0:1, :], AF.Ln)
        nc.scalar.activation(rcpt_row[0:1, :], lt_row[0:1, :],
                             AF.Exp, scale=-0.5, bias=lnb[0:1, 0:1])
        nc.sync.dma_start(cbuf_n[0:1, :], rcpt_row[0:1, :])
        if no_collective:
            for r in range(n_cores):
                nc.sync.dma_start(
                    rcpT[:, r * (rows // P):(r + 1) * (rows // P)],
                    cbuf_n[0:1, :].rearrange("a (x p) -> (a p) x", p=P))
        else:
            nc.gpsimd.collective_compute(
                "AllGather", ALU.bypass,
                replica_groups=[list(range(n_cores))],
                ins=[cbuf_n[:].opt()], outs=[cbuf_n_out[:].opt()])
            nc.sync.dma_start(
                rcpT[:],
                cbuf_n_out[0:1, :].rearrange("a (x p) -> (a p) x", p=P))

        # --- Phase A2: img norms (broadcast via ones-matmul), fp8 normalize --
        ssq_img = ex_ps.tile([P, rows], F32, tag="ex")
        for k in range(kt):
            sq = sqp.tile([P, rows], BF16, tag="sq")
            nc.vector.tensor_tensor(sq[:], img_h[:, k, :], img_h[:, k, :],
                                    ALU.mult)
            for h in range(rows // 512):
                nc.tensor.matmul(ssq_img[:, h * 512:(h + 1) * 512],
                                 ones_bsb[:], sq[:, h * 512:(h + 1) * 512],
                                 start=(k == 0), stop=(k == kt - 1))
        nc.scalar.activation(li_bc[:], ssq_img[:], AF.Ln)
        nc.scalar.activation(rimg_bc[:], li_bc[:], AF.Exp, scale=-0.5)
        for k in range(kt):
            nc.vector.tensor_tensor(imgT8[:, k, :], img_h[:, k, :],
                                    rimg_bc[:], ALU.mult)

        # diag dots (for Draw'): d-major elementwise + ones-matmul reduction
        dg_ps = ex_ps.tile([P, rows], F32, tag="ex")
        for k in range(kt):
            dsc = sqp.tile([P, rows], BF16, tag="sq")
            nc.vector.tensor_tensor(dsc[:], img_h[:, k, :], to_h[:, k, :],
                                    ALU.mult)
            for h in range(rows // 512):
                nc.tensor.matmul(dg_ps[:, h * 512:(h + 1) * 512],
                                 ones_bsb[:], dsc[:, h * 512:(h + 1) * 512],
                                 start=(k == 0), stop=(k == kt - 1))
        w1 = v1.tile([1, rows], F32, tag="w1")
        nc.vector.tensor_tensor(w1[0:1, :], dg_ps[0:1, :], rimg_bc[0:1, :],
                                ALU.mult)
        w2 = v1.tile([1, rows], F32, tag="w2")
        nc.vector.tensor_tensor(w2[0:1, :], w1[0:1, :], rcpt_row[0:1, :],
                                ALU.mult)
        nc.vector.tensor_reduce(sc[0:1, 1:2], w2[0:1, :],
                                axis=mybir.AxisListType.X, op=ALU.add)

        if stop_after == "A":
            nc.sync.dma_start(out[0:1, 0:1], sc[0:1, 1:2])
            return

        # --- Phase C: main fp8 DoubleRow matmul + exp + reductions -----------
        HB = rows // 512                     # img halves per chunk (2)
        hch = n_ch // 2
        for c in range(n_ch):
            mm = ex_ps.tile([P, rows], F32, tag="ex")
            for t in range(kt // 2):
                for h in range(HB):
                    nc.tensor.matmul(
                        mm[:, h * 512:(h + 1) * 512],
                        txtT8[:, 2 * t:2 * t + 2, c * P:(c + 1) * P],
                        imgT8[:, 2 * t:2 * t + 2, h * 512:(h + 1) * 512],
                        start=(t == 0), stop=(t == kt // 2 - 1),
                        perf_mode=DR)
            ex = exp_p.tile([P, rows], BF16, tag="exp")
            nc.scalar.activation(ex[:], mm[:], AF.Exp,
                                 bias=ebias[:, 0:1], scale=rcpT[:, c:c + 1],
                                 accum_out=csacc[:, c:c + 1])
            if c == 0:
                nc.vector.tensor_copy(racc[:], ex[:])
            else:
                nc.vector.tensor_tensor(racc[:], racc[:], ex[:], ALU.add)
            if c == hch - 1:
                # first half of colsums complete: overlap its AllReduce with
                # the second half of the GEMM
                nc.sync.dma_start(
                    cbuf1[0:1, :].rearrange("a (x p) -> (a p) x", p=P),
                    csacc[:, 0:hch])
                if no_collective:
                    nc.sync.dma_start(cbuf1_out[:], cbuf1[:])
                else:
                    nc.gpsimd.collective_compute(
                        "AllReduce", ALU.add,
                        replica_groups=[list(range(n_cores))],
                        ins=[cbuf1[:].opt()], outs=[cbuf1_out[:].opt()])

        if stop_after == "C":
            nc.sync.dma_start(out[0:1, 0:1], csacc[0:1, 0:1])
            return

        # --- Phase D: local scalars ------------------------------------------
        # R_m = sum_i ln(rowsum_i): partition-reduce racc via ones-matmul
        for h in range(HB):
            rs = ex_ps.tile([P, rows], F32, tag="ex")
            nc.tensor.matmul(rs[0:1, 0:512], ones_bsb[:, 0:1],
                             racc[:, h * 512:(h + 1) * 512],
                             start=True, stop=True)
            lnr = v1.tile([P, 512], BF16, tag="lnr")
            nc.scalar.activation(lnr[0:1, :], rs[0:1, 0:512], AF.Ln,
                                 accum_out=sc[0:1, 2 + h:3 + h])
        nc.vector.tensor_tensor(sc[0:1, 0:1], sc[0:1, 2:3], sc[0:1, 3:4],
                                ALU.add)                         # R_m

        # ship partials: [colsums second half (4096), R_m, Draw'_m]
        nc.sync.dma_start(
            cbuf2[0:1, 0:half].rearrange("a (x p) -> (a p) x", p=P),
            csacc[:, hch:n_ch])
        nc.sync.dma_start(cbuf2[0:1, half:half + 2], sc[0:1, 0:2])

        if stop_after == "D":
            nc.sync.dma_start(out[0:1, 0:1], sc[0:1, 0:1])
            return

        # --- Phase E: second AllReduce + finish ------------------------------
        if no_collective:
            nc.sync.dma_start(cbuf2_out[:], cbuf2[:])
        else:
            nc.gpsimd.collective_compute(
                "AllReduce", ALU.add,
                replica_groups=[list(range(n_cores))],
                ins=[cbuf2[:].opt()], outs=[cbuf2_out[:].opt()])

        # ln of global colsums (first half overlaps the GEMM already)
        nc.sync.dma_start(
            cs_sb[:, 0:hch],
            cbuf1_out[0:1, :].rearrange("a (x p) -> (a p) x", p=P))
        lacc = v1.tile([P, 8], F32, tag="v1")
        nc.scalar.activation(ln_cs[:, 0:hch], cs_sb[:, 0:hch], AF.Ln,
                             accum_out=lacc[:, 0:1])
        nc.sync.dma_start(
            cs_sb[:, hch:n_ch],
            cbuf2_out[0:1, 0:half].rearrange("a (x p) -> (a p) x", p=P))
        nc.scalar.activation(ln_cs[:, hch:n_ch], cs_sb[:, hch:n_ch], AF.Ln,
                             accum_out=lacc[:, 1:2])
        nc.vector.tensor_tensor(lacc[:, 2:3], lacc[:, 0:1], lacc[:, 1:2],
                                ALU.add)
        lps = ex_ps.tile([P, rows], F32, tag="ex")
        nc.tensor.matmul(lps[0:1, 0:1], ones_sb[:, 0:1], lacc[:, 2:3],
                         start=True, stop=True)                  # L
        rd = v1.tile([P, 8], F32, tag="v1")
        nc.sync.dma_start(rd[0:1, 0:2], cbuf2_out[0:1, half:half + 2])

        # loss = cexp + (R + L - 2 * Draw') / (2N)
        fin = v1.tile([P, 8], F32, tag="v1")
        nc.vector.tensor_tensor(fin[0:1, 0:1], rd[0:1, 0:1], lps[0:1, 0:1],
                                ALU.add)                         # R + L
        nc.vector.tensor_scalar_mul(fin[0:1, 1:2], rd[0:1, 1:2], -2.0)
        nc.vector.tensor_tensor(fin[0:1, 2:3], fin[0:1, 0:1], fin[0:1, 1:2],
                                ALU.add)
        nc.scalar.activation(fin[0:1, 3:4], fin[0:1, 2:3], AF.Copy,
                             bias=float(cexp), scale=float(1.0 / (2 * n)))
        nc.sync.dma_start(out[0:1, 0:1], fin[0:1, 3:4])


def make_in_maps(image_features, text_features, n=N, d=D, n_cores=N_CORES):
    image_features = np.asarray(image_features, dtype=np.float32)
    text_features = np.asarray(text_features, dtype=np.float32)
    rows = n // n_cores
    txt_t8 = np.ascontiguousarray(text_features.T).astype(ml_dtypes.float8_e4m3)
    ones = np.ones((128, 128), dtype=np.float32)
    ones_b = np.ones((128, 128), dtype=ml_dtypes.bfloat16)
    return [
        {
            "img_t": np.ascontiguousarray(
                image_features[m * rows:(m + 1) * rows].T),
            "txt_t8": txt_t8,
            "to_t": np.ascontiguousarray(
                text_features[m * rows:(m + 1) * rows].T),
            "ones": ones,
            "ones_b": ones_b,
        }
        for m in range(n_cores)
    ]


_CACHE = {}
_LOCK = threading.Lock()


def _get_nc():
    with _LOCK:
        if "nc" not in _CACHE:
            _CACHE["nc"] = build_nc()
        return _CACHE["nc"]


def kernel(image_features, text_features):
    image_features = np.asarray(image_features, dtype=np.float32)
    text_features = np.asarray(text_features, dtype=np.float32)
    assert image_features.shape == (N, D) and text_features.shape == (N, D)
    nc = _get_nc()
    in_maps = make_in_maps(image_features, text_features)
    res = run_bass_kernel_spmd(nc, in_maps, list(range(N_CORES)))
    val = np.float32(res.results[0]["out"][0, 0])
    return np.array(val, dtype=np.float32)
